# revision 59
# baseline (speedup 1.0000x reference)
"""ColBERT-style max-sim retrieval kernel for 8 Trainium2 NeuronCores.

Math (reference):
    scores[q,d,t,l] = sum_e doc[d,t,e] * query[q,e,l]
    out[q,d] = sum_l max_t scores[q,d,t,l]

Shapes (hardcoded): doc_tokens [128,128,128] f32, query_tokens [128,128,32] f32,
out [128,128] f32.

Sharding: data-parallel over the query batch dim Nq across the 8 cores
(16 queries per core); doc_tokens replicated to every core. Each core
computes its [16, 128] slab of the output independently; the host
concatenates the slabs.

Per-core dataflow:
  - PE transposes every doc [t,e] -> [e,t] (transpose-mode matmul with an
    identity), ScalarE copies PSUM->SBUF into a resident docT[e, d, t].
  - Main matmuls in float32r (full PE rate at N=512): lhsT = 4 queries x 32
    tokens = [e,128], rhs = 4-doc slab of docT = [e,512]; PSUM holds
    scores[(q,l), (d,t)].
  - VectorE segmented reduce_max over t: [128, 8, 128] -> [128, 8].
  - One fp32 matmul with a block-diagonal ones matrix sums over l
    (partition-axis reduction), then DMA out.
"""

import numpy as np

import concourse.bass as bass
import concourse.tile as tile
from concourse import masks, mybir
from concourse.bass_utils import run_bass_kernel_spmd
from concourse.vector_clock import ScopedClock

N_CORES = 8
ND, LD, E = 128, 128, 128      # docs, doc tokens, embed dim
NQ, LQ = 128, 32               # queries, query tokens
NQC = NQ // N_CORES            # queries per core = 16
QG = 4                         # queries per matmul M-group (4*32 = 128 = M)
NG = NQC // QG                 # M-groups per core = 4
F32 = mybir.dt.float32
F32R = mybir.dt.float32r

# walrus in this container rejects multiple sem waits on a single
# instruction (varies by opcode template; 1 is safe everywhere); split a
# Tile-assigned instruction's waits across carrier instructions.
_MAX_DRAIN_WAITS = 1


def _patched_drain_and_barrier(self, tick_clock, wait_clock):
    nc = self.nc
    drain_inst = nc.sync.drain()
    wait_clock.add_sem_waits(
        drain_inst.ins, ScopedClock({None: tick_clock.global_clock})
    )
    si = drain_inst.ins.sync_info
    waits = list(si.on_wait) if si is not None and si.on_wait else []
    if len(waits) > _MAX_DRAIN_WAITS:
        si.on_wait = waits[:_MAX_DRAIN_WAITS]
        drain_inst.ins.sync_info = si
        rest = waits[_MAX_DRAIN_WAITS:]
        while rest:
            extra = nc.sync.drain()
            esi = extra.ins.sync_info
            if esi is None:
                esi = si
            esi.on_wait = rest[:_MAX_DRAIN_WAITS]
            esi.on_update = []
            extra.ins.sync_info = esi
            rest = rest[_MAX_DRAIN_WAITS:]
    nc.all_engine_barrier()
    assert self.sems is not None
    popped = nc._tile_sem_poison_stack.pop()
    assert popped is self._sem_poison
    nc.clear_and_free_semaphores(list(self.sems.allocated().values()))
    nc.all_engine_barrier()


def _apply_tile_patch():
    if getattr(tile.TileContext, "_drain_patch_applied", False):
        return
    tile.TileContext._drain_and_barrier = _patched_drain_and_barrier
    tile.TileContext._drain_patch_applied = True


def _split_excess_waits(nc, max_waits=_MAX_DRAIN_WAITS):
    """walrus rejects instructions with too many sem waits (2 for most
    opcodes, 1 for matmul whose waits land on the single-slot LDWEIGHTS
    struct); move the excess onto NoOp carriers inserted immediately before
    the instruction on the same engine (same-engine program order makes
    this semantically identical)."""
    for f in nc.m.functions:
        for blk in f.blocks:
            snapshot = list(blk.instructions)
            for idx in range(len(snapshot) - 1, -1, -1):
                inst = snapshot[idx]
                limit = max_waits
                si = getattr(inst, "sync_info", None)
                if si is None or not si.on_wait or len(si.on_wait) <= limit:
                    continue
                waits = list(si.on_wait)
                si.on_wait = waits[-limit:]
                inst.sync_info = si
                rest = waits[:-limit]
                chunks = [
                    rest[i : i + max_waits] for i in range(0, len(rest), max_waits)
                ]
                for chunk in reversed(chunks):
                    noop = mybir.InstNoOp(
                        name=nc.get_next_instruction_name(),
                        engine=inst.engine,
                        bass_nofuse=True,
                    )
                    noop.sync_info = mybir.SyncInfo(on_wait=chunk, on_update=[])
                    nc.register_instruction(noop)
                    blk.instructions.insert(idx, noop)


def _emit_quarter_sum(nc, qtr, lsum, maxq, pt_pool, osb_pool, out_dram):
    NQTR = ND // 4
    psum_out = pt_pool.tile([QG, NG, NQTR], F32, tag="pt")
    nc.tensor.matmul(psum_out[:], lsum[:], maxq[qtr][:])
    outsb = osb_pool.tile([QG, NG, NQTR], F32, tag="osb")
    nc.scalar.copy(outsb[:], psum_out[:])
    out_view = out_dram[:].rearrange("(g qi) d -> qi g d", qi=QG)
    nc.sync.dma_start(
        out_view[:, :, qtr * NQTR : (qtr + 1) * NQTR], outsb[:]
    )


def _build_nc():
    _apply_tile_patch()
    nc = bass.Bass("TRN2", target_bir_lowering=False, debug=False)
    doc_dram = nc.dram_tensor("doc_tokens", [ND, LD, E], F32, kind="ExternalInput")
    q_dram = nc.dram_tensor("query_tokens", [NQC, E, LQ], F32, kind="ExternalInput")
    out_dram = nc.dram_tensor("out", [NQC, ND], F32, kind="ExternalOutput")

    with tile.TileContext(nc) as tc:
        with (
            tc.tile_pool(name="const", bufs=1) as const_pool,
            tc.tile_pool(name="docT", bufs=1) as docT_pool,
            tc.tile_pool(name="stage", bufs=4) as stage_pool,
            tc.tile_pool(name="acc", bufs=1) as acc_pool,
            tc.tile_pool(name="osb", bufs=2) as osb_pool,
            tc.tile_pool(name="pt", bufs=2, space="PSUM") as pt_pool,
            tc.tile_pool(name="ps", bufs=2, space="PSUM") as ps_pool,
        ):
            # Constants: identity for PE transpose, block-diagonal ones for
            # the final sum-over-l matmul.
            identity = const_pool.tile([128, 128], F32)
            masks.make_identity(nc, identity[:])
            lsum = const_pool.tile([128, QG], F32)
            nc.gpsimd.memset(lsum[:], 0.0)
            for m in range(QG):
                nc.gpsimd.memset(lsum[32 * m : 32 * (m + 1), m : m + 1], 1.0)

            # All 16 queries, laid out [e, q_local, l] so that a 4-query
            # slice is a ready-made matmul lhsT of [K=128, M=128]. The f32r
            # matmul requires operands produced as rounded f32r, so DMA to a
            # staging tile and round-cast with ScalarE. Issued on the
            # scalar-engine HWDGE ring so the doc-block DMAs (SP ring) are
            # not queued behind it at startup.
            qstage = const_pool.tile([E, NQC, LQ], F32)
            nc.scalar.dma_start(qstage[:], q_dram[:].rearrange("q e l -> e q l"))
            qsb = const_pool.tile([E, NQC, LQ], F32R)
            nc.scalar.copy(qsb[:], qstage[:])

            # Resident transposed docs: docT[e, d, t] (f32r for the main MMs).
            docT = docT_pool.tile([E, ND, LD], F32R)
            # Per-group running max_t: [(q,l), g, d], physically split into
            # four doc quarters so each quarter's sum-over-l matmul can fire
            # as soon as its docs are done without read/write conflicts
            # against later blocks.
            NQTR = ND // 4  # 32 docs per quarter
            maxq = []
            for q in range(4):
                mq = acc_pool.tile([128, NG, NQTR], F32, tag=f"mq{q}", name=f"mq{q}")
                maxq.append(mq)

            # PE warm-up: dependency-free transposes of the identity keep
            # the PE busy from t~0 so the HAM clock gate is released before
            # the first real matmuls arrive (and they cost nothing — PE
            # would idle during the first doc DMA anyway).
            warm = pt_pool.tile([128, LD], F32, tag="pt")
            for _ in range(12):
                nc.tensor.transpose(warm[:], identity[:], identity[:])

            # Doc blocks of 12/12/8 per 32-doc quarter: 12-doc (3-PSUM-bank)
            # score tiles amortize the DVE reduce's 120-cycle PSUM-access
            # overhead over more elements, and blocks never straddle a
            # quarter boundary.
            blocks = []
            for q in range(4):
                blocks += [(32 * q, 8), (32 * q + 8, 12), (32 * q + 20, 12)]

            doc_view = doc_dram[:].rearrange("d t e -> t d e")
            for bi, (d0, size) in enumerate(blocks):
                stage = stage_pool.tile([LD, 12, E], F32, tag="stage")
                if bi == 0:
                    # Split DMA so the first transposes start a transfer
                    # earlier.
                    nc.sync.dma_start(
                        stage[:, 0:4, :], doc_view[:, d0 : d0 + 4, :]
                    )
                    nc.sync.dma_start(
                        stage[:, 4:size, :], doc_view[:, d0 + 4 : d0 + size, :]
                    )
                else:
                    nc.sync.dma_start(
                        stage[:, 0:size, :], doc_view[:, d0 : d0 + size, :]
                    )

                if bi == 0:
                    # Startup fast path: the PE queue is FIFO, so emit
                    # transpose -> copy -> matmuls -> reduce per 4-doc half
                    # to get the first DVE reduce going as early as possible.
                    for lo in range(0, size, 4):
                        psumT = pt_pool.tile([128, 4, LD], F32, tag="pt")
                        for j in range(4):
                            nc.tensor.transpose(
                                psumT[:, j, :], stage[:, lo + j, :], identity[:]
                            )
                        nc.scalar.copy(
                            docT[:, d0 + lo : d0 + lo + 4, :], psumT[:]
                        )
                        for g in range(NG):
                            lhsT = qsb[:, QG * g : QG * (g + 1), :]
                            psumS = ps_pool.tile([128, 4, LD], F32, tag="ps")
                            nc.tensor.matmul(
                                psumS[:],
                                lhsT,
                                docT[:, d0 + lo : d0 + lo + 4, :],
                            )
                            nc.vector.reduce_max(
                                maxq[0][:, g, lo : lo + 4],
                                psumS[:],
                                axis=mybir.AxisListType.X,
                            )
                    continue

                # Transpose the block's docs in 4-doc PSUM batches, ScalarE
                # copies them out to docT.
                for lo in range(0, size, 4):
                    psumT = pt_pool.tile([128, 4, LD], F32, tag="pt")
                    for j in range(4):
                        nc.tensor.transpose(
                            psumT[:, j, :], stage[:, lo + j, :], identity[:]
                        )
                    nc.scalar.copy(
                        docT[:, d0 + lo : d0 + lo + 4, :], psumT[:]
                    )

                # Scores + segmented max over t for each query group (DVE's
                # 1x-rate tensor_reduce from PSUM is the kernel bottleneck).
                qtr = d0 // NQTR
                c0 = d0 % NQTR
                for g in range(NG):
                    lhsT = qsb[:, QG * g : QG * (g + 1), :]
                    psumS = ps_pool.tile([128, size, LD], F32, tag="ps")
                    for m0 in range(0, size, 4):
                        nc.tensor.matmul(
                            psumS[:, m0 : m0 + 4, :],
                            lhsT,
                            docT[:, d0 + m0 : d0 + m0 + 4, :],
                        )
                    nc.vector.reduce_max(
                        maxq[qtr][:, g, c0 : c0 + size],
                        psumS[:],
                        axis=mybir.AxisListType.X,
                    )

                # Sum over l (partition-axis, groups of 32) via one fp32
                # matmul per completed doc quarter, emitted 1-2 blocks after
                # the quarter's last reduce so the DVE pipeline has drained
                # and PE/ACT/SP head-of-line blocking is negligible:
                # out[qi, (g, d)] = sum_p lsum[p, qi] * maxq[p, (g, d)].
                if bi in (4, 7, 10):
                    _emit_quarter_sum(
                        nc, {4: 0, 7: 1, 10: 2}[bi], lsum, maxq, pt_pool,
                        osb_pool, out_dram,
                    )
            _emit_quarter_sum(nc, 3, lsum, maxq, pt_pool, osb_pool, out_dram)

    _split_excess_waits(nc)
    return nc


_NC_CACHE = None


def _get_nc():
    global _NC_CACHE
    if _NC_CACHE is None:
        _NC_CACHE = _build_nc()
    return _NC_CACHE


def kernel(doc_tokens, query_tokens):
    doc_tokens = np.ascontiguousarray(np.asarray(doc_tokens, dtype=np.float32))
    query_tokens = np.ascontiguousarray(np.asarray(query_tokens, dtype=np.float32))
    assert doc_tokens.shape == (ND, LD, E), doc_tokens.shape
    assert query_tokens.shape == (NQ, E, LQ), query_tokens.shape

    nc = _get_nc()
    in_maps = [
        {
            "doc_tokens": doc_tokens,
            "query_tokens": query_tokens[c * NQC : (c + 1) * NQC],
        }
        for c in range(N_CORES)
    ]
    res = run_bass_kernel_spmd(nc, in_maps, list(range(N_CORES))).results
    return np.concatenate([res[c]["out"] for c in range(N_CORES)], axis=0)


# revision 65
# speedup vs baseline: 1.0004x; 1.0004x over previous
"""ColBERT-style max-sim retrieval kernel for 8 Trainium2 NeuronCores.

Math (reference):
    scores[q,d,t,l] = sum_e doc[d,t,e] * query[q,e,l]
    out[q,d] = sum_l max_t scores[q,d,t,l]

Shapes (hardcoded): doc_tokens [128,128,128] f32, query_tokens [128,128,32] f32,
out [128,128] f32.

Sharding: data-parallel over the query batch dim Nq across the 8 cores
(16 queries per core); doc_tokens replicated to every core. Each core
computes its [16, 128] slab of the output independently; the host
concatenates the slabs.

Per-core dataflow:
  - PE transposes every doc [t,e] -> [e,t] (transpose-mode matmul with an
    identity), ScalarE copies PSUM->SBUF into a resident docT[e, d, t].
  - Main matmuls in float32r (full PE rate at N=512): lhsT = 4 queries x 32
    tokens = [e,128], rhs = 4-doc slab of docT = [e,512]; PSUM holds
    scores[(q,l), (d,t)].
  - VectorE segmented reduce_max over t: [128, 8, 128] -> [128, 8].
  - One fp32 matmul with a block-diagonal ones matrix sums over l
    (partition-axis reduction), then DMA out.
"""

import numpy as np

import concourse.bass as bass
import concourse.tile as tile
from concourse import masks, mybir
from concourse.bass_utils import run_bass_kernel_spmd
from concourse.vector_clock import ScopedClock

N_CORES = 8
ND, LD, E = 128, 128, 128      # docs, doc tokens, embed dim
NQ, LQ = 128, 32               # queries, query tokens
NQC = NQ // N_CORES            # queries per core = 16
QG = 4                         # queries per matmul M-group (4*32 = 128 = M)
NG = NQC // QG                 # M-groups per core = 4
F32 = mybir.dt.float32
F32R = mybir.dt.float32r

# walrus in this container rejects multiple sem waits on a single
# instruction (varies by opcode template; 1 is safe everywhere); split a
# Tile-assigned instruction's waits across carrier instructions.
_MAX_DRAIN_WAITS = 1


def _patched_drain_and_barrier(self, tick_clock, wait_clock):
    nc = self.nc
    drain_inst = nc.sync.drain()
    wait_clock.add_sem_waits(
        drain_inst.ins, ScopedClock({None: tick_clock.global_clock})
    )
    si = drain_inst.ins.sync_info
    waits = list(si.on_wait) if si is not None and si.on_wait else []
    if len(waits) > _MAX_DRAIN_WAITS:
        si.on_wait = waits[:_MAX_DRAIN_WAITS]
        drain_inst.ins.sync_info = si
        rest = waits[_MAX_DRAIN_WAITS:]
        while rest:
            extra = nc.sync.drain()
            esi = extra.ins.sync_info
            if esi is None:
                esi = si
            esi.on_wait = rest[:_MAX_DRAIN_WAITS]
            esi.on_update = []
            extra.ins.sync_info = esi
            rest = rest[_MAX_DRAIN_WAITS:]
    nc.all_engine_barrier()
    assert self.sems is not None
    popped = nc._tile_sem_poison_stack.pop()
    assert popped is self._sem_poison
    nc.clear_and_free_semaphores(list(self.sems.allocated().values()))
    nc.all_engine_barrier()


def _apply_tile_patch():
    if getattr(tile.TileContext, "_drain_patch_applied", False):
        return
    tile.TileContext._drain_and_barrier = _patched_drain_and_barrier
    tile.TileContext._drain_patch_applied = True


def _split_excess_waits(nc, max_waits=_MAX_DRAIN_WAITS):
    """walrus rejects instructions with too many sem waits (2 for most
    opcodes, 1 for matmul whose waits land on the single-slot LDWEIGHTS
    struct); move the excess onto NoOp carriers inserted immediately before
    the instruction on the same engine (same-engine program order makes
    this semantically identical)."""
    for f in nc.m.functions:
        for blk in f.blocks:
            snapshot = list(blk.instructions)
            for idx in range(len(snapshot) - 1, -1, -1):
                inst = snapshot[idx]
                limit = max_waits
                si = getattr(inst, "sync_info", None)
                if si is None or not si.on_wait or len(si.on_wait) <= limit:
                    continue
                waits = list(si.on_wait)
                si.on_wait = waits[-limit:]
                inst.sync_info = si
                rest = waits[:-limit]
                chunks = [
                    rest[i : i + max_waits] for i in range(0, len(rest), max_waits)
                ]
                for chunk in reversed(chunks):
                    noop = mybir.InstNoOp(
                        name=nc.get_next_instruction_name(),
                        engine=inst.engine,
                        bass_nofuse=True,
                    )
                    noop.sync_info = mybir.SyncInfo(on_wait=chunk, on_update=[])
                    nc.register_instruction(noop)
                    blk.instructions.insert(idx, noop)


def _emit_quarter_sum(nc, qtr, lsum, maxq, pt_pool, osb_pool, out_dram):
    NQTR = ND // 4
    psum_out = pt_pool.tile([QG, NG, NQTR], F32, tag="pt")
    nc.tensor.matmul(psum_out[:], lsum[:], maxq[qtr][:])
    outsb = osb_pool.tile([QG, NG, NQTR], F32, tag="osb")
    nc.scalar.copy(outsb[:], psum_out[:])
    out_view = out_dram[:].rearrange("(g qi) d -> qi g d", qi=QG)
    nc.sync.dma_start(
        out_view[:, :, qtr * NQTR : (qtr + 1) * NQTR], outsb[:]
    )


def _build_nc():
    _apply_tile_patch()
    nc = bass.Bass("TRN2", target_bir_lowering=False, debug=False)
    doc_dram = nc.dram_tensor("doc_tokens", [ND, LD, E], F32, kind="ExternalInput")
    q_dram = nc.dram_tensor("query_tokens", [NQC, E, LQ], F32, kind="ExternalInput")
    out_dram = nc.dram_tensor("out", [NQC, ND], F32, kind="ExternalOutput")

    with tile.TileContext(nc) as tc:
        with (
            tc.tile_pool(name="const", bufs=1) as const_pool,
            tc.tile_pool(name="docT", bufs=1) as docT_pool,
            tc.tile_pool(name="stage", bufs=4) as stage_pool,
            tc.tile_pool(name="acc", bufs=1) as acc_pool,
            tc.tile_pool(name="osb", bufs=2) as osb_pool,
            tc.tile_pool(name="pt", bufs=2, space="PSUM") as pt_pool,
            tc.tile_pool(name="ps", bufs=2, space="PSUM") as ps_pool,
        ):
            # Constants: identity for PE transpose, block-diagonal ones for
            # the final sum-over-l matmul.
            identity = const_pool.tile([128, 128], F32)
            masks.make_identity(nc, identity[:])
            lsum = const_pool.tile([128, QG], F32)
            nc.gpsimd.memset(lsum[:], 0.0)
            for m in range(QG):
                nc.gpsimd.memset(lsum[32 * m : 32 * (m + 1), m : m + 1], 1.0)

            # All 16 queries, laid out [e, q_local, l] so that a 4-query
            # slice is a ready-made matmul lhsT of [K=128, M=128]. The f32r
            # matmul requires operands produced as rounded f32r, so DMA to a
            # staging tile and round-cast with ScalarE. Issued on the
            # scalar-engine HWDGE ring so the doc-block DMAs (SP ring) are
            # not queued behind it at startup.
            q_view = q_dram[:].rearrange("q e l -> e q l")
            qstage = const_pool.tile([E, NQC, LQ], F32)
            qsb = const_pool.tile([E, NQC, LQ], F32R)
            nc.scalar.dma_start(qstage[:], q_view[:])

            # Resident transposed docs: docT[e, d, t] (f32r for the main MMs).
            docT = docT_pool.tile([E, ND, LD], F32R)
            # Per-group running max_t: [(q,l), g, d], physically split into
            # four doc quarters so each quarter's sum-over-l matmul can fire
            # as soon as its docs are done without read/write conflicts
            # against later blocks.
            NQTR = ND // 4  # 32 docs per quarter
            maxq = []
            for q in range(4):
                mq = acc_pool.tile([128, NG, NQTR], F32, tag=f"mq{q}", name=f"mq{q}")
                maxq.append(mq)

            # PE warm-up: dependency-free transposes of the identity keep
            # the PE busy from t~0 so the HAM clock gate is released before
            # the first real matmuls arrive (and they cost nothing — PE
            # would idle during the first doc DMA anyway).
            warm = pt_pool.tile([128, LD], F32, tag="pt")
            for _ in range(10):
                nc.tensor.transpose(warm[:], identity[:], identity[:])

            # Doc blocks of 12/12/8 per 32-doc quarter: 12-doc (3-PSUM-bank)
            # score tiles amortize the DVE reduce's 120-cycle PSUM-access
            # overhead over more elements, and blocks never straddle a
            # quarter boundary.
            blocks = []
            for q in range(4):
                blocks += [(32 * q, 8), (32 * q + 8, 12), (32 * q + 20, 12)]

            doc_view = doc_dram[:].rearrange("d t e -> t d e")
            for bi, (d0, size) in enumerate(blocks):
                stage = stage_pool.tile([LD, 12, E], F32, tag="stage")
                if bi == 0:
                    # Split DMA so the first transposes start a transfer
                    # earlier.
                    nc.sync.dma_start(
                        stage[:, 0:4, :], doc_view[:, d0 : d0 + 4, :]
                    )
                    nc.sync.dma_start(
                        stage[:, 4:size, :], doc_view[:, d0 + 4 : d0 + size, :]
                    )
                else:
                    nc.sync.dma_start(
                        stage[:, 0:size, :], doc_view[:, d0 : d0 + size, :]
                    )

                if bi == 0:
                    # Startup fast path: the PE queue is FIFO, so emit
                    # transpose -> copy -> matmuls -> reduce per 4-doc half
                    # to get the first DVE reduce going as early as possible.
                    for lo in range(0, size, 4):
                        psumT = pt_pool.tile([128, 4, LD], F32, tag="pt")
                        for j in range(4):
                            nc.tensor.transpose(
                                psumT[:, j, :], stage[:, lo + j, :], identity[:]
                            )
                        nc.scalar.copy(
                            docT[:, d0 + lo : d0 + lo + 4, :], psumT[:]
                        )
                        for g in range(NG):
                            if lo == 0 and g == 0:
                                # Round-cast the queries to f32r on the DVE
                                # (idle at startup; fp32 SBUF copy gets 2x
                                # mode there) so ScalarE's in-order queue
                                # stays clear for the docT copies.
                                nc.vector.tensor_copy(qsb[:], qstage[:])
                            lhsT = qsb[:, QG * g : QG * (g + 1), :]
                            psumS = ps_pool.tile([128, 4, LD], F32, tag="ps")
                            nc.tensor.matmul(
                                psumS[:],
                                lhsT,
                                docT[:, d0 + lo : d0 + lo + 4, :],
                            )
                            nc.vector.reduce_max(
                                maxq[0][:, g, lo : lo + 4],
                                psumS[:],
                                axis=mybir.AxisListType.X,
                            )
                    continue

                # Transpose the block's docs in 4-doc PSUM batches, ScalarE
                # copies them out to docT.
                for lo in range(0, size, 4):
                    psumT = pt_pool.tile([128, 4, LD], F32, tag="pt")
                    for j in range(4):
                        nc.tensor.transpose(
                            psumT[:, j, :], stage[:, lo + j, :], identity[:]
                        )
                    nc.scalar.copy(
                        docT[:, d0 + lo : d0 + lo + 4, :], psumT[:]
                    )

                # Scores + segmented max over t for each query group (DVE's
                # 1x-rate tensor_reduce from PSUM is the kernel bottleneck).
                qtr = d0 // NQTR
                c0 = d0 % NQTR
                for g in range(NG):
                    lhsT = qsb[:, QG * g : QG * (g + 1), :]
                    psumS = ps_pool.tile([128, size, LD], F32, tag="ps")
                    for m0 in range(0, size, 4):
                        nc.tensor.matmul(
                            psumS[:, m0 : m0 + 4, :],
                            lhsT,
                            docT[:, d0 + m0 : d0 + m0 + 4, :],
                        )
                    nc.vector.reduce_max(
                        maxq[qtr][:, g, c0 : c0 + size],
                        psumS[:],
                        axis=mybir.AxisListType.X,
                    )

                # Sum over l (partition-axis, groups of 32) via one fp32
                # matmul per completed doc quarter, emitted 1-2 blocks after
                # the quarter's last reduce so the DVE pipeline has drained
                # and PE/ACT/SP head-of-line blocking is negligible:
                # out[qi, (g, d)] = sum_p lsum[p, qi] * maxq[p, (g, d)].
                if bi in (4, 7, 10):
                    _emit_quarter_sum(
                        nc, {4: 0, 7: 1, 10: 2}[bi], lsum, maxq, pt_pool,
                        osb_pool, out_dram,
                    )
            _emit_quarter_sum(nc, 3, lsum, maxq, pt_pool, osb_pool, out_dram)

    _split_excess_waits(nc)
    return nc


_NC_CACHE = None


def _get_nc():
    global _NC_CACHE
    if _NC_CACHE is None:
        _NC_CACHE = _build_nc()
    return _NC_CACHE


def kernel(doc_tokens, query_tokens):
    doc_tokens = np.ascontiguousarray(np.asarray(doc_tokens, dtype=np.float32))
    query_tokens = np.ascontiguousarray(np.asarray(query_tokens, dtype=np.float32))
    assert doc_tokens.shape == (ND, LD, E), doc_tokens.shape
    assert query_tokens.shape == (NQ, E, LQ), query_tokens.shape

    nc = _get_nc()
    in_maps = [
        {
            "doc_tokens": doc_tokens,
            "query_tokens": query_tokens[c * NQC : (c + 1) * NQC],
        }
        for c in range(N_CORES)
    ]
    res = run_bass_kernel_spmd(nc, in_maps, list(range(N_CORES))).results
    return np.concatenate([res[c]["out"] for c in range(N_CORES)], axis=0)


# revision 66
# speedup vs baseline: 1.0056x; 1.0052x over previous
"""ColBERT-style max-sim retrieval kernel for 8 Trainium2 NeuronCores.

Math (reference):
    scores[q,d,t,l] = sum_e doc[d,t,e] * query[q,e,l]
    out[q,d] = sum_l max_t scores[q,d,t,l]

Shapes (hardcoded): doc_tokens [128,128,128] f32, query_tokens [128,128,32] f32,
out [128,128] f32.

Sharding: data-parallel over the query batch dim Nq across the 8 cores
(16 queries per core); doc_tokens replicated to every core. Each core
computes its [16, 128] slab of the output independently; the host
concatenates the slabs.

Per-core dataflow:
  - PE transposes every doc [t,e] -> [e,t] (transpose-mode matmul with an
    identity), ScalarE copies PSUM->SBUF into a resident docT[e, d, t].
  - Main matmuls in float32r (full PE rate at N=512): lhsT = 4 queries x 32
    tokens = [e,128], rhs = 4-doc slab of docT = [e,512]; PSUM holds
    scores[(q,l), (d,t)].
  - VectorE segmented reduce_max over t: [128, 8, 128] -> [128, 8].
  - One fp32 matmul with a block-diagonal ones matrix sums over l
    (partition-axis reduction), then DMA out.
"""

import numpy as np

import concourse.bass as bass
import concourse.tile as tile
from concourse import masks, mybir
from concourse.bass_utils import run_bass_kernel_spmd
from concourse.vector_clock import ScopedClock

N_CORES = 8
ND, LD, E = 128, 128, 128      # docs, doc tokens, embed dim
NQ, LQ = 128, 32               # queries, query tokens
NQC = NQ // N_CORES            # queries per core = 16
QG = 4                         # queries per matmul M-group (4*32 = 128 = M)
NG = NQC // QG                 # M-groups per core = 4
F32 = mybir.dt.float32
F32R = mybir.dt.float32r

# walrus in this container rejects multiple sem waits on a single
# instruction (varies by opcode template; 1 is safe everywhere); split a
# Tile-assigned instruction's waits across carrier instructions.
_MAX_DRAIN_WAITS = 1


def _patched_drain_and_barrier(self, tick_clock, wait_clock):
    nc = self.nc
    drain_inst = nc.sync.drain()
    wait_clock.add_sem_waits(
        drain_inst.ins, ScopedClock({None: tick_clock.global_clock})
    )
    si = drain_inst.ins.sync_info
    waits = list(si.on_wait) if si is not None and si.on_wait else []
    if len(waits) > _MAX_DRAIN_WAITS:
        si.on_wait = waits[:_MAX_DRAIN_WAITS]
        drain_inst.ins.sync_info = si
        rest = waits[_MAX_DRAIN_WAITS:]
        while rest:
            extra = nc.sync.drain()
            esi = extra.ins.sync_info
            if esi is None:
                esi = si
            esi.on_wait = rest[:_MAX_DRAIN_WAITS]
            esi.on_update = []
            extra.ins.sync_info = esi
            rest = rest[_MAX_DRAIN_WAITS:]
    nc.all_engine_barrier()
    assert self.sems is not None
    popped = nc._tile_sem_poison_stack.pop()
    assert popped is self._sem_poison
    nc.clear_and_free_semaphores(list(self.sems.allocated().values()))
    nc.all_engine_barrier()


def _apply_tile_patch():
    if getattr(tile.TileContext, "_drain_patch_applied", False):
        return
    tile.TileContext._drain_and_barrier = _patched_drain_and_barrier
    tile.TileContext._drain_patch_applied = True


def _split_excess_waits(nc, max_waits=_MAX_DRAIN_WAITS):
    """walrus rejects instructions with too many sem waits (2 for most
    opcodes, 1 for matmul whose waits land on the single-slot LDWEIGHTS
    struct); move the excess onto NoOp carriers inserted immediately before
    the instruction on the same engine (same-engine program order makes
    this semantically identical)."""
    for f in nc.m.functions:
        for blk in f.blocks:
            snapshot = list(blk.instructions)
            for idx in range(len(snapshot) - 1, -1, -1):
                inst = snapshot[idx]
                limit = max_waits
                si = getattr(inst, "sync_info", None)
                if si is None or not si.on_wait or len(si.on_wait) <= limit:
                    continue
                waits = list(si.on_wait)
                si.on_wait = waits[-limit:]
                inst.sync_info = si
                rest = waits[:-limit]
                chunks = [
                    rest[i : i + max_waits] for i in range(0, len(rest), max_waits)
                ]
                for chunk in reversed(chunks):
                    noop = mybir.InstNoOp(
                        name=nc.get_next_instruction_name(),
                        engine=inst.engine,
                        bass_nofuse=True,
                    )
                    noop.sync_info = mybir.SyncInfo(on_wait=chunk, on_update=[])
                    nc.register_instruction(noop)
                    blk.instructions.insert(idx, noop)


def _emit_quarter_sum(nc, qtr, lsum, maxq, pt_pool, osb_pool, out_dram):
    NQTR = ND // 4
    psum_out = pt_pool.tile([QG, NG, NQTR], F32, tag="pt")
    nc.tensor.matmul(psum_out[:], lsum[:], maxq[qtr][:])
    outsb = osb_pool.tile([QG, NG, NQTR], F32, tag="osb")
    nc.scalar.copy(outsb[:], psum_out[:])
    out_view = out_dram[:].rearrange("(g qi) d -> qi g d", qi=QG)
    nc.sync.dma_start(
        out_view[:, :, qtr * NQTR : (qtr + 1) * NQTR], outsb[:]
    )


def _build_nc():
    _apply_tile_patch()
    nc = bass.Bass("TRN2", target_bir_lowering=False, debug=False)
    doc_dram = nc.dram_tensor("doc_tokens", [ND, LD, E], F32, kind="ExternalInput")
    q_dram = nc.dram_tensor("query_tokens", [NQC, E, LQ], F32, kind="ExternalInput")
    out_dram = nc.dram_tensor("out", [NQC, ND], F32, kind="ExternalOutput")

    with tile.TileContext(nc) as tc:
        with (
            tc.tile_pool(name="const", bufs=1) as const_pool,
            tc.tile_pool(name="docT", bufs=1) as docT_pool,
            tc.tile_pool(name="stage", bufs=4) as stage_pool,
            tc.tile_pool(name="acc", bufs=1) as acc_pool,
            tc.tile_pool(name="osb", bufs=2) as osb_pool,
            tc.tile_pool(name="pt", bufs=2, space="PSUM") as pt_pool,
            tc.tile_pool(name="ps", bufs=2, space="PSUM") as ps_pool,
        ):
            # Constants: identity for PE transpose, block-diagonal ones for
            # the final sum-over-l matmul.
            identity = const_pool.tile([128, 128], F32)
            masks.make_identity(nc, identity[:])
            lsum = const_pool.tile([128, QG], F32)
            nc.gpsimd.memset(lsum[:], 0.0)
            for m in range(QG):
                nc.gpsimd.memset(lsum[32 * m : 32 * (m + 1), m : m + 1], 1.0)

            # All 16 queries, laid out [e, q_local, l] so that a 4-query
            # slice is a ready-made matmul lhsT of [K=128, M=128]. The f32r
            # matmul requires operands produced as rounded f32r, so DMA to a
            # staging tile and round-cast with ScalarE. Issued on the
            # scalar-engine HWDGE ring so the doc-block DMAs (SP ring) are
            # not queued behind it at startup.
            q_view = q_dram[:].rearrange("q e l -> e q l")
            qstage = const_pool.tile([E, NQC, LQ], F32)
            qsb = const_pool.tile([E, NQC, LQ], F32R)
            nc.scalar.dma_start(qstage[:], q_view[:])

            # Resident transposed docs: docT[e, d, t] (f32r for the main MMs).
            docT = docT_pool.tile([E, ND, LD], F32R)
            # Per-group running max_t: [(q,l), g, d], physically split into
            # four doc quarters so each quarter's sum-over-l matmul can fire
            # as soon as its docs are done without read/write conflicts
            # against later blocks.
            NQTR = ND // 4  # 32 docs per quarter
            maxq = []
            for q in range(4):
                mq = acc_pool.tile([128, NG, NQTR], F32, tag=f"mq{q}", name=f"mq{q}")
                maxq.append(mq)

            # PE warm-up: dependency-free transposes of the identity keep
            # the PE busy from t~0 so the HAM clock gate is released before
            # the first real matmuls arrive (and they cost nothing — PE
            # would idle during the first doc DMA anyway).
            warm = pt_pool.tile([128, LD], F32, tag="pt")
            for _ in range(10):
                nc.tensor.transpose(warm[:], identity[:], identity[:])

            # Doc blocks of 12/12/8 per 32-doc quarter: 12-doc (3-PSUM-bank)
            # score tiles amortize the DVE reduce's 120-cycle PSUM-access
            # overhead over more elements, and blocks never straddle a
            # quarter boundary.
            blocks = []
            for q in range(4):
                blocks += [(32 * q, 8), (32 * q + 8, 12), (32 * q + 20, 12)]

            doc_view = doc_dram[:].rearrange("d t e -> t d e")
            for bi, (d0, size) in enumerate(blocks):
                stage = stage_pool.tile([LD, 12, E], F32, tag="stage")
                if bi <= 2:
                    # Ramp-up blocks: split the DMA so the first transpose
                    # batch starts a transfer earlier (the serialized DMA
                    # queue otherwise gates the whole block on its last
                    # byte).
                    nc.sync.dma_start(
                        stage[:, 0:4, :], doc_view[:, d0 : d0 + 4, :]
                    )
                    nc.sync.dma_start(
                        stage[:, 4:size, :], doc_view[:, d0 + 4 : d0 + size, :]
                    )
                else:
                    nc.sync.dma_start(
                        stage[:, 0:size, :], doc_view[:, d0 : d0 + size, :]
                    )

                if bi == 0:
                    # Startup fast path: the PE queue is FIFO, so emit
                    # transpose -> copy -> matmuls -> reduce per 4-doc half
                    # to get the first DVE reduce going as early as possible.
                    for lo in range(0, size, 4):
                        psumT = pt_pool.tile([128, 4, LD], F32, tag="pt")
                        for j in range(4):
                            nc.tensor.transpose(
                                psumT[:, j, :], stage[:, lo + j, :], identity[:]
                            )
                        nc.scalar.copy(
                            docT[:, d0 + lo : d0 + lo + 4, :], psumT[:]
                        )
                        for g in range(NG):
                            if lo == 0 and g == 0:
                                # Round-cast the queries to f32r on the DVE
                                # (idle at startup; fp32 SBUF copy gets 2x
                                # mode there) so ScalarE's in-order queue
                                # stays clear for the docT copies.
                                nc.vector.tensor_copy(qsb[:], qstage[:])
                            lhsT = qsb[:, QG * g : QG * (g + 1), :]
                            psumS = ps_pool.tile([128, 4, LD], F32, tag="ps")
                            nc.tensor.matmul(
                                psumS[:],
                                lhsT,
                                docT[:, d0 + lo : d0 + lo + 4, :],
                            )
                            nc.vector.reduce_max(
                                maxq[0][:, g, lo : lo + 4],
                                psumS[:],
                                axis=mybir.AxisListType.X,
                            )
                    continue

                # Transpose the block's docs in 4-doc PSUM batches, ScalarE
                # copies them out to docT.
                for lo in range(0, size, 4):
                    psumT = pt_pool.tile([128, 4, LD], F32, tag="pt")
                    for j in range(4):
                        nc.tensor.transpose(
                            psumT[:, j, :], stage[:, lo + j, :], identity[:]
                        )
                    nc.scalar.copy(
                        docT[:, d0 + lo : d0 + lo + 4, :], psumT[:]
                    )

                # Scores + segmented max over t for each query group (DVE's
                # 1x-rate tensor_reduce from PSUM is the kernel bottleneck).
                qtr = d0 // NQTR
                c0 = d0 % NQTR
                for g in range(NG):
                    lhsT = qsb[:, QG * g : QG * (g + 1), :]
                    psumS = ps_pool.tile([128, size, LD], F32, tag="ps")
                    for m0 in range(0, size, 4):
                        nc.tensor.matmul(
                            psumS[:, m0 : m0 + 4, :],
                            lhsT,
                            docT[:, d0 + m0 : d0 + m0 + 4, :],
                        )
                    nc.vector.reduce_max(
                        maxq[qtr][:, g, c0 : c0 + size],
                        psumS[:],
                        axis=mybir.AxisListType.X,
                    )

                # Sum over l (partition-axis, groups of 32) via one fp32
                # matmul per completed doc quarter, emitted 1-2 blocks after
                # the quarter's last reduce so the DVE pipeline has drained
                # and PE/ACT/SP head-of-line blocking is negligible:
                # out[qi, (g, d)] = sum_p lsum[p, qi] * maxq[p, (g, d)].
                if bi in (4, 7, 10):
                    _emit_quarter_sum(
                        nc, {4: 0, 7: 1, 10: 2}[bi], lsum, maxq, pt_pool,
                        osb_pool, out_dram,
                    )
            _emit_quarter_sum(nc, 3, lsum, maxq, pt_pool, osb_pool, out_dram)

    _split_excess_waits(nc)
    return nc


_NC_CACHE = None


def _get_nc():
    global _NC_CACHE
    if _NC_CACHE is None:
        _NC_CACHE = _build_nc()
    return _NC_CACHE


def kernel(doc_tokens, query_tokens):
    doc_tokens = np.ascontiguousarray(np.asarray(doc_tokens, dtype=np.float32))
    query_tokens = np.ascontiguousarray(np.asarray(query_tokens, dtype=np.float32))
    assert doc_tokens.shape == (ND, LD, E), doc_tokens.shape
    assert query_tokens.shape == (NQ, E, LQ), query_tokens.shape

    nc = _get_nc()
    in_maps = [
        {
            "doc_tokens": doc_tokens,
            "query_tokens": query_tokens[c * NQC : (c + 1) * NQC],
        }
        for c in range(N_CORES)
    ]
    res = run_bass_kernel_spmd(nc, in_maps, list(range(N_CORES))).results
    return np.concatenate([res[c]["out"] for c in range(N_CORES)], axis=0)
